# revision 3
# baseline (speedup 1.0000x reference)
"""Cumulative (running) group norm over the frame axis on 8 trn2 NeuronCores.

Input  x: [B=8, T=8192, C=512] f32, weight: [C] f32.
Sharding: data-parallel over B -> one example per core, SPMD (identical
program, per-core input slice). No cross-core communication.

Per-core algorithm (T=8192 frames, C=512 channels), all math in f32, with a
chunk-blocked partition-major time layout: a chunk of W columns covers
frames [start, start + 128*W), frame t = start + p*W + j (p = partition,
j = column in chunk). Each chunk load/store then moves W*2KiB CONTIGUOUS
bytes per partition (measured ~8% faster HBM streaming than the 2KiB
interleaved layout).

Per chunk:
  - per-frame channel mean/var via bn_stats/bn_aggr -> mu[p,j], v[p,j]
  - cumsum over t:  cs = per-partition inclusive scan along j
    (tensor_tensor_scan); off[m] = sum_{p<m} tot[p] (strict-triu matmul);
    chunk total broadcast via all-ones matmul; carry accumulates across
    chunks.  running mean m = (cs + off + carry) * (1/(t+1)).
  - q = v + (mu - m)^2, same scan machinery -> running var.
  - out = (x - m) * 1/sqrt(var + eps) [* weight]   (ACT Identity with
    per-partition scale/bias, one instruction per column).
The chunk plan tapers (…,4,2,1,1) so the final store is ready quickly
after the final load, minimizing the DMA tail.

Measured (marginal For_i loop timing, 8 cores concurrent): ~105 us/core
vs ~102 us for a pure-DMA loop moving the same 32 MiB — ~3% from the
hardware's streaming floor. (HBM per-core limit ~350 GB/s; 16 MiB in +
16 MiB out is the irreducible traffic.)
"""

from contextlib import ExitStack

import numpy as np

import concourse.bacc as bacc
import concourse.bass as bass
import concourse.tile as tile
from concourse import mybir
from concourse.bass_utils import run_bass_kernel_spmd

B, T, C = 8, 8192, 512
P = 128            # SBUF partitions
NT = T // P        # 64 frame-columns per core
EPS = 1e-5
F32 = mybir.dt.float32
ADD = mybir.AluOpType.add
MULT = mybir.AluOpType.mult

# Chunk widths (columns per chunk); sum must be 64. Tapered tail.
PLAN = [8, 8, 8, 8, 8, 8, 8, 4, 2, 1, 1]
assert sum(PLAN) == 64

# Loads round-robin on the two HWDGE queues (SP, ACT); stores on the SWDGE
# (Pool) queue so a pending store never head-of-line blocks a load.
LOAD_ENGS = ["sync", "scalar"]
STORE_ENGS = ["gpsimd"]
# stage C: even columns on ACT (Identity w/ scale+bias), odd columns on DVE
# (tensor_scalar mult+add) — the two engines normalize a chunk concurrently,
# shortening the store-ready lag (measured ~2 us/iter over all-ACT).
STAGEC = "split"


def _emit_consts(nc, tc, ctx, striu_in, ones_in, invc_in, w_in):
    singles = ctx.enter_context(tc.tile_pool(name="singles", bufs=1))
    striu = singles.tile([P, P], F32)
    ones = singles.tile([P, P], F32)
    invc = singles.tile([P, NT], F32)
    invc_neg = singles.tile([P, NT], F32)
    nc.gpsimd.dma_start(out=striu, in_=striu_in[:, :])
    nc.gpsimd.dma_start(out=ones, in_=ones_in[:, :])
    nc.gpsimd.dma_start(out=invc, in_=invc_in[:, :])
    nc.vector.tensor_scalar_mul(invc_neg, invc, -1.0)
    wb = None
    if w_in is not None:
        wb = singles.tile([P, C], F32)
        nc.gpsimd.dma_start(out=wb, in_=w_in[:].to_broadcast((P, C)))
    eps_t = singles.tile([P, 1], F32)
    nc.vector.memset(eps_t, EPS)
    return striu, ones, invc, invc_neg, wb, eps_t


def _emit_body(nc, tc, ctx, x_in, out_ext, consts, uid=""):
    """Emit one full normalization pass x_in -> out_ext (DRAM APs)."""
    striu, ones, invc, invc_neg, wb, eps_t = consts

    ld_engs = [getattr(nc, e) for e in LOAD_ENGS]
    st_engs = [getattr(nc, e) for e in STORE_ENGS]
    ld_i = [0]
    st_i = [0]

    def next_ld():
        e = ld_engs[ld_i[0] % len(ld_engs)]
        ld_i[0] += 1
        return e

    def next_st():
        e = st_engs[st_i[0] % len(st_engs)]
        st_i[0] += 1
        return e

    big = ctx.enter_context(tc.tile_pool(name=f"big{uid}", bufs=1))
    stats = ctx.enter_context(tc.tile_pool(name=f"stats{uid}", bufs=8))
    mvs = ctx.enter_context(tc.tile_pool(name=f"mvs{uid}", bufs=2))
    sm = ctx.enter_context(tc.tile_pool(name=f"sm{uid}", bufs=3))
    psum = ctx.enter_context(tc.tile_pool(name=f"psum{uid}", bufs=2, space="PSUM"))

    xb = big.tile([P, NT, C], F32)

    zero4 = sm.tile([P, 4], F32)
    nc.vector.memset(zero4, 0.0)
    zero = zero4[:, 0:1]
    carry_mu = zero4[:, 1:2]
    carry_q = zero4[:, 2:3]

    c0 = 0
    for ch, W in enumerate(PLAN):
        nfr = P * W
        start = c0 * P  # frames before this chunk
        # ---- stage A: load chunk + per-frame stats --------------------
        rows = x_in[start:start + nfr, :]
        next_ld().dma_start(
            out=xb[:, c0:c0 + W, :],
            in_=rows.rearrange("(p j) c -> p j c", p=P),
        )
        mv = mvs.tile([P, W, 2], F32)
        for j in range(W):
            st = stats.tile([P, 6], F32)
            nc.vector.bn_stats(out=st, in_=xb[:, c0 + j, :])
            nc.vector.bn_aggr(out=mv[:, j, :], in_=st)

        # ---- stage B: running stats over time -------------------------
        mu = sm.tile([P, W], F32)
        vv = sm.tile([P, W], F32)
        nc.vector.tensor_copy(out=mu, in_=mv[:, :, 0])
        nc.vector.tensor_copy(out=vv, in_=mv[:, :, 1])

        def running(vals, carry):
            """-> (stot [P,W] running total through frame t, new carry)."""
            cs = sm.tile([P, W], F32)
            nc.vector.tensor_tensor_scan(
                cs, ones[:, :W], vals, zero, MULT, ADD)
            tot = cs[:, W - 1:W]
            off_p = psum.tile([P, 1], F32)
            chtot_p = psum.tile([P, 1], F32)
            nc.tensor.matmul(off_p, striu, tot, start=True, stop=True)
            nc.tensor.matmul(chtot_p, ones, tot, start=True, stop=True)
            offc = sm.tile([P, 1], F32)
            nc.vector.tensor_add(out=offc, in0=off_p, in1=carry)
            stot = sm.tile([P, W], F32)
            nc.vector.tensor_scalar_add(stot, cs, offc)
            ncarry = sm.tile([P, 1], F32)
            nc.vector.tensor_add(out=ncarry, in0=chtot_p, in1=carry)
            return stot, ncarry

        stot, carry_mu = running(mu, carry_mu)
        # m_neg = -running_mean (via negated invcnt): the Identity
        # activation bias needs no extra negation op.
        m_neg = sm.tile([P, W], F32)
        nc.vector.tensor_mul(out=m_neg, in0=stot, in1=invc_neg[:, c0:c0 + W])

        d = sm.tile([P, W], F32)
        q = sm.tile([P, W], F32)
        nc.vector.tensor_add(out=d, in0=mu, in1=m_neg)
        nc.vector.tensor_mul(out=q, in0=d, in1=d)
        nc.vector.tensor_add(out=q, in0=q, in1=vv)

        vtot, carry_q = running(q, carry_q)
        var = sm.tile([P, W], F32)
        nc.vector.tensor_mul(out=var, in0=vtot, in1=invc[:, c0:c0 + W])

        rstd = sm.tile([P, W], F32)
        nc.scalar.activation(
            out=rstd, in_=var, func=mybir.ActivationFunctionType.Sqrt,
            bias=eps_t[:, 0:1])
        nc.vector.reciprocal(out=rstd, in_=rstd)
        nmr = sm.tile([P, W], F32)
        nc.vector.tensor_mul(out=nmr, in0=m_neg, in1=rstd)

        # ---- stage C: normalize + store -------------------------------
        for j in range(W):
            if STAGEC == "split" and (j % 2 == 1):
                nc.vector.tensor_scalar(
                    out=xb[:, c0 + j, :], in0=xb[:, c0 + j, :],
                    scalar1=rstd[:, j:j + 1], scalar2=nmr[:, j:j + 1],
                    op0=MULT, op1=ADD)
            else:
                nc.scalar.activation(
                    out=xb[:, c0 + j, :], in_=xb[:, c0 + j, :],
                    func=mybir.ActivationFunctionType.Identity,
                    bias=nmr[:, j:j + 1], scale=rstd[:, j:j + 1])
            if wb is not None:
                nc.vector.tensor_mul(
                    out=xb[:, c0 + j, :], in0=xb[:, c0 + j, :], in1=wb)
        orows = out_ext[start:start + nfr, :]
        next_st().dma_start(
            out=orows.rearrange("(p j) c -> p j c", p=P),
            in_=xb[:, c0:c0 + W, :],
        )
        c0 += W


def _consts() -> dict[str, np.ndarray]:
    striu = np.triu(np.ones((P, P), dtype=np.float32), k=1)
    ones = np.ones((P, P), dtype=np.float32)
    invcnt = np.zeros((P, NT), dtype=np.float32)
    c0 = 0
    for W in PLAN:
        start = c0 * P
        for j in range(W):
            t = start + np.arange(P, dtype=np.float32) * W + j
            invcnt[:, c0 + j] = 1.0 / (t + 1.0)
        c0 += W
    return {"striu": striu, "ones": ones, "invcnt": invcnt}


def _declare(nc, apply_weight):
    x_in = nc.declare_dram_parameter("x", [T, C], F32, isOutput=False)
    striu_in = nc.declare_dram_parameter("striu", [P, P], F32, isOutput=False)
    ones_in = nc.declare_dram_parameter("ones", [P, P], F32, isOutput=False)
    invc_in = nc.declare_dram_parameter("invcnt", [P, NT], F32, isOutput=False)
    w_in = None
    if apply_weight:
        w_in = nc.declare_dram_parameter("weight", [C], F32, isOutput=False)
    out_ext = nc.declare_dram_parameter("out", [T, C], F32, isOutput=True)
    return x_in, striu_in, ones_in, invc_in, w_in, out_ext


def _build(apply_weight: bool) -> bass.Bass:
    nc = bacc.Bacc(None, target_bir_lowering=False, debug=False)
    x_in, striu_in, ones_in, invc_in, w_in, out_ext = _declare(nc, apply_weight)
    with tile.TileContext(nc) as tc, ExitStack() as ctx:
        consts = _emit_consts(nc, tc, ctx, striu_in, ones_in, invc_in, w_in)
        _emit_body(nc, tc, ctx, x_in, out_ext, consts)
    nc.compile()
    return nc


def _build_loop_timing(k_iters: int) -> bass.Bass:
    """Timing-only: tiny I/O; k_iters For_i loop normalizing an internal
    DRAM buffer in place. Marginal wall time between two k values isolates
    pure per-iteration HW execution."""
    nc = bacc.Bacc(None, target_bir_lowering=False, debug=False)
    striu_in = nc.declare_dram_parameter("striu", [P, P], F32, isOutput=False)
    ones_in = nc.declare_dram_parameter("ones", [P, P], F32, isOutput=False)
    invc_in = nc.declare_dram_parameter("invcnt", [P, NT], F32, isOutput=False)
    out_ext = nc.declare_dram_parameter("out", [P, 4], F32, isOutput=True)
    d = nc.dram_tensor("dwork", [T, C], F32)

    with tile.TileContext(nc) as tc, ExitStack() as octx:
        consts = _emit_consts(nc, tc, octx, striu_in, ones_in, invc_in, None)
        with tc.For_i(0, k_iters, 1):
            with ExitStack() as ictx:
                _emit_body(nc, tc, ictx, d, d, consts, uid="_L")
        nc.sync.dma_start(out=out_ext[:, :], in_=d[0:P, 0:4])
    nc.compile()
    return nc


_PROGRAMS: dict[bool, bass.Bass] = {}


def _run(inputs: dict, **run_kwargs):
    x = np.ascontiguousarray(np.asarray(inputs["x"], dtype=np.float32))
    w = np.ascontiguousarray(np.asarray(inputs["weight"], dtype=np.float32))
    apply_weight = not bool(np.all(w == 1.0))
    if apply_weight not in _PROGRAMS:
        _PROGRAMS[apply_weight] = _build(apply_weight)
    nc = _PROGRAMS[apply_weight]
    consts = _consts()
    in_maps = []
    for b in range(B):
        m = {"x": x[b], **consts}
        if apply_weight:
            m["weight"] = w
        in_maps.append(m)
    res = run_bass_kernel_spmd(nc, in_maps, core_ids=list(range(B)),
                               **run_kwargs)
    out = np.stack([res.results[b]["out"] for b in range(B)], axis=0)
    return out, res


def kernel(**inputs) -> np.ndarray:
    in_dtype = np.asarray(inputs["x"]).dtype
    out, _ = _run(inputs)
    return out.astype(in_dtype)


# revision 4
# speedup vs baseline: 1.0012x; 1.0012x over previous
"""Cumulative (running) group norm over the frame axis on 8 trn2 NeuronCores.

Input  x: [B=8, T=8192, C=512] f32, weight: [C] f32.
Sharding: data-parallel over B -> one example per core, SPMD (identical
program, per-core input slice). No cross-core communication.

Per-core algorithm (T=8192 frames, C=512 channels), all math in f32, with a
chunk-blocked partition-major time layout: a chunk of W columns covers
frames [start, start + 128*W), frame t = start + p*W + j (p = partition,
j = column in chunk). Each chunk load/store then moves W*2KiB CONTIGUOUS
bytes per partition (measured ~8% faster HBM streaming than the 2KiB
interleaved layout).

Per chunk:
  - per-frame channel mean/var via bn_stats/bn_aggr -> mu[p,j], v[p,j]
  - cumsum over t:  cs = per-partition inclusive scan along j
    (tensor_tensor_scan); off[m] = sum_{p<m} tot[p] (strict-triu matmul);
    chunk total broadcast via all-ones matmul; carry accumulates across
    chunks.  running mean m = (cs + off + carry) * (1/(t+1)).
  - q = v + (mu - m)^2, same scan machinery -> running var.
  - out = (x - m) * 1/sqrt(var + eps) [* weight]   (ACT Identity with
    per-partition scale/bias, one instruction per column).
The chunk plan tapers (…,4,2,1,1) so the final store is ready quickly
after the final load, minimizing the DMA tail.

Measured (marginal For_i loop timing, 8 cores concurrent): ~105 us/core
vs ~102 us for a pure-DMA loop moving the same 32 MiB — ~3% from the
hardware's streaming floor. (HBM per-core limit ~350 GB/s; 16 MiB in +
16 MiB out is the irreducible traffic.)
"""

from contextlib import ExitStack

import numpy as np

import concourse.bacc as bacc
import concourse.bass as bass
import concourse.tile as tile
from concourse import mybir
from concourse.bass_utils import run_bass_kernel_spmd

B, T, C = 8, 8192, 512
P = 128            # SBUF partitions
NT = T // P        # 64 frame-columns per core
EPS = 1e-5
F32 = mybir.dt.float32
ADD = mybir.AluOpType.add
MULT = mybir.AluOpType.mult

# Chunk widths (columns per chunk); sum must be 64. Tapered tail.
PLAN = [8, 8, 8, 8, 8, 8, 8, 4, 2, 1, 1]
assert sum(PLAN) == 64

# Loads round-robin on the two HWDGE queues (SP, ACT); stores on the SWDGE
# (Pool) queue so a pending store never head-of-line blocks a load.
LOAD_ENGS = ["sync", "scalar"]
STORE_ENGS = ["gpsimd"]
# stage C: even columns on ACT (Identity w/ scale+bias), odd columns on DVE
# (tensor_scalar mult+add) — the two engines normalize a chunk concurrently,
# shortening the store-ready lag (measured ~2 us/iter over all-ACT).
STAGEC = "split"


def _emit_consts(nc, tc, ctx, striu_in, ones_in, invc_in, w_in):
    singles = ctx.enter_context(tc.tile_pool(name="singles", bufs=1))
    striu = singles.tile([P, P], F32)
    ones = singles.tile([P, P], F32)
    invc = singles.tile([P, NT], F32)
    invc_neg = singles.tile([P, NT], F32)
    nc.gpsimd.dma_start(out=striu, in_=striu_in[:, :])
    nc.gpsimd.dma_start(out=ones, in_=ones_in[:, :])
    nc.gpsimd.dma_start(out=invc, in_=invc_in[:, :])
    nc.vector.tensor_scalar_mul(invc_neg, invc, -1.0)
    wb = None
    if w_in is not None:
        wb = singles.tile([P, C], F32)
        nc.gpsimd.dma_start(out=wb, in_=w_in[:].to_broadcast((P, C)))
    eps_t = singles.tile([P, 1], F32)
    nc.vector.memset(eps_t, EPS)
    return striu, ones, invc, invc_neg, wb, eps_t


def _emit_body(nc, tc, ctx, x_in, out_ext, consts, uid=""):
    """Emit one full normalization pass x_in -> out_ext (DRAM APs)."""
    striu, ones, invc, invc_neg, wb, eps_t = consts

    ld_engs = [getattr(nc, e) for e in LOAD_ENGS]
    st_engs = [getattr(nc, e) for e in STORE_ENGS]
    ld_i = [0]
    st_i = [0]

    def next_ld():
        e = ld_engs[ld_i[0] % len(ld_engs)]
        ld_i[0] += 1
        return e

    def next_st():
        e = st_engs[st_i[0] % len(st_engs)]
        st_i[0] += 1
        return e

    big = ctx.enter_context(tc.tile_pool(name=f"big{uid}", bufs=1))
    stats = ctx.enter_context(tc.tile_pool(name=f"stats{uid}", bufs=8))
    mvs = ctx.enter_context(tc.tile_pool(name=f"mvs{uid}", bufs=2))
    sm = ctx.enter_context(tc.tile_pool(name=f"sm{uid}", bufs=3))
    psum = ctx.enter_context(tc.tile_pool(name=f"psum{uid}", bufs=2, space="PSUM"))

    xb = big.tile([P, NT, C], F32)

    zero4 = sm.tile([P, 4], F32)
    nc.vector.memset(zero4, 0.0)
    zero = zero4[:, 0:1]
    carry_mu = zero4[:, 1:2]
    carry_q = zero4[:, 2:3]

    c0 = 0
    for ch, W in enumerate(PLAN):
        nfr = P * W
        start = c0 * P  # frames before this chunk
        # ---- stage A: load chunk + per-frame stats --------------------
        rows = x_in[start:start + nfr, :]
        next_ld().dma_start(
            out=xb[:, c0:c0 + W, :],
            in_=rows.rearrange("(p j) c -> p j c", p=P),
        )
        mv = mvs.tile([P, W, 2], F32)
        for j in range(W):
            st = stats.tile([P, 6], F32)
            nc.vector.bn_stats(out=st, in_=xb[:, c0 + j, :])
            nc.vector.bn_aggr(out=mv[:, j, :], in_=st)

        # ---- stage B: running stats over time -------------------------
        mu = sm.tile([P, W], F32)
        vv = sm.tile([P, W], F32)
        nc.vector.tensor_copy(out=mu, in_=mv[:, :, 0])
        nc.vector.tensor_copy(out=vv, in_=mv[:, :, 1])

        def running(vals, carry):
            """-> (stot [P,W] running total through frame t, new carry)."""
            cs = sm.tile([P, W], F32)
            nc.vector.tensor_tensor_scan(
                cs, ones[:, :W], vals, zero, MULT, ADD)
            tot = cs[:, W - 1:W]
            off_p = psum.tile([P, 1], F32)
            chtot_p = psum.tile([P, 1], F32)
            nc.tensor.matmul(off_p, striu, tot, start=True, stop=True)
            nc.tensor.matmul(chtot_p, ones, tot, start=True, stop=True)
            offc = sm.tile([P, 1], F32)
            nc.vector.tensor_add(out=offc, in0=off_p, in1=carry)
            stot = sm.tile([P, W], F32)
            nc.vector.tensor_scalar_add(stot, cs, offc)
            ncarry = sm.tile([P, 1], F32)
            nc.vector.tensor_add(out=ncarry, in0=chtot_p, in1=carry)
            return stot, ncarry

        stot, carry_mu = running(mu, carry_mu)
        # m_neg = -running_mean (via negated invcnt): the Identity
        # activation bias needs no extra negation op.
        m_neg = sm.tile([P, W], F32)
        nc.vector.tensor_mul(out=m_neg, in0=stot, in1=invc_neg[:, c0:c0 + W])

        d = sm.tile([P, W], F32)
        q = sm.tile([P, W], F32)
        nc.vector.tensor_add(out=d, in0=mu, in1=m_neg)
        nc.vector.tensor_mul(out=q, in0=d, in1=d)
        nc.vector.tensor_add(out=q, in0=q, in1=vv)

        vtot, carry_q = running(q, carry_q)
        var = sm.tile([P, W], F32)
        nc.vector.tensor_mul(out=var, in0=vtot, in1=invc[:, c0:c0 + W])

        rstd = sm.tile([P, W], F32)
        nc.scalar.activation(
            out=rstd, in_=var, func=mybir.ActivationFunctionType.Sqrt,
            bias=eps_t[:, 0:1])
        nc.vector.reciprocal(out=rstd, in_=rstd)
        nmr = sm.tile([P, W], F32)
        nc.vector.tensor_mul(out=nmr, in0=m_neg, in1=rstd)

        # ---- stage C: normalize + store -------------------------------
        for j in range(W):
            if STAGEC == "split" and (j % 2 == 1):
                nc.vector.tensor_scalar(
                    out=xb[:, c0 + j, :], in0=xb[:, c0 + j, :],
                    scalar1=rstd[:, j:j + 1], scalar2=nmr[:, j:j + 1],
                    op0=MULT, op1=ADD)
            else:
                nc.scalar.activation(
                    out=xb[:, c0 + j, :], in_=xb[:, c0 + j, :],
                    func=mybir.ActivationFunctionType.Identity,
                    bias=nmr[:, j:j + 1], scale=rstd[:, j:j + 1])
            if wb is not None:
                nc.vector.tensor_mul(
                    out=xb[:, c0 + j, :], in0=xb[:, c0 + j, :], in1=wb)
        orows = out_ext[start:start + nfr, :]
        next_st().dma_start(
            out=orows.rearrange("(p j) c -> p j c", p=P),
            in_=xb[:, c0:c0 + W, :],
        )
        c0 += W


def _consts() -> dict[str, np.ndarray]:
    striu = np.triu(np.ones((P, P), dtype=np.float32), k=1)
    ones = np.ones((P, P), dtype=np.float32)
    invcnt = np.zeros((P, NT), dtype=np.float32)
    c0 = 0
    for W in PLAN:
        start = c0 * P
        for j in range(W):
            t = start + np.arange(P, dtype=np.float32) * W + j
            invcnt[:, c0 + j] = 1.0 / (t + 1.0)
        c0 += W
    return {"striu": striu, "ones": ones, "invcnt": invcnt}


def _declare(nc, apply_weight):
    x_in = nc.declare_dram_parameter("x", [T, C], F32, isOutput=False)
    striu_in = nc.declare_dram_parameter("striu", [P, P], F32, isOutput=False)
    ones_in = nc.declare_dram_parameter("ones", [P, P], F32, isOutput=False)
    invc_in = nc.declare_dram_parameter("invcnt", [P, NT], F32, isOutput=False)
    w_in = None
    if apply_weight:
        w_in = nc.declare_dram_parameter("weight", [C], F32, isOutput=False)
    out_ext = nc.declare_dram_parameter("out", [T, C], F32, isOutput=True)
    return x_in, striu_in, ones_in, invc_in, w_in, out_ext


def _build(apply_weight: bool) -> bass.Bass:
    nc = bacc.Bacc(None, target_bir_lowering=False, debug=False)
    x_in, striu_in, ones_in, invc_in, w_in, out_ext = _declare(nc, apply_weight)
    with tile.TileContext(nc) as tc, ExitStack() as ctx:
        consts = _emit_consts(nc, tc, ctx, striu_in, ones_in, invc_in, w_in)
        _emit_body(nc, tc, ctx, x_in, out_ext, consts)
    nc.compile()
    return nc


def _build_loop_timing(k_iters: int) -> bass.Bass:
    """Timing-only: tiny I/O; k_iters For_i loop normalizing an internal
    DRAM buffer in place. Marginal wall time between two k values isolates
    pure per-iteration HW execution."""
    nc = bacc.Bacc(None, target_bir_lowering=False, debug=False)
    striu_in = nc.declare_dram_parameter("striu", [P, P], F32, isOutput=False)
    ones_in = nc.declare_dram_parameter("ones", [P, P], F32, isOutput=False)
    invc_in = nc.declare_dram_parameter("invcnt", [P, NT], F32, isOutput=False)
    out_ext = nc.declare_dram_parameter("out", [P, 4], F32, isOutput=True)
    d = nc.dram_tensor("dwork", [T, C], F32)

    with tile.TileContext(nc) as tc, ExitStack() as octx:
        consts = _emit_consts(nc, tc, octx, striu_in, ones_in, invc_in, None)
        with tc.For_i(0, k_iters, 1):
            with ExitStack() as ictx:
                _emit_body(nc, tc, ictx, d, d, consts, uid="_L")
        nc.sync.dma_start(out=out_ext[:, :], in_=d[0:P, 0:4])
    nc.compile()
    return nc


def _build_loop_timing_pp(k_iters: int) -> bass.Bass:
    """Ping-pong timing loop: each iteration runs two passes d->e, e->d so
    no pass reads the DRAM buffer it just wrote (matches the real kernel's
    distinct in/out buffers). Per-pass time = marginal / 2."""
    nc = bacc.Bacc(None, target_bir_lowering=False, debug=False)
    striu_in = nc.declare_dram_parameter("striu", [P, P], F32, isOutput=False)
    ones_in = nc.declare_dram_parameter("ones", [P, P], F32, isOutput=False)
    invc_in = nc.declare_dram_parameter("invcnt", [P, NT], F32, isOutput=False)
    out_ext = nc.declare_dram_parameter("out", [P, 4], F32, isOutput=True)
    d = nc.dram_tensor("dwork", [T, C], F32)
    e = nc.dram_tensor("dwork2", [T, C], F32)

    with tile.TileContext(nc) as tc, ExitStack() as octx:
        consts = _emit_consts(nc, tc, octx, striu_in, ones_in, invc_in, None)
        with tc.For_i(0, k_iters, 1):
            with ExitStack() as ictx:
                _emit_body(nc, tc, ictx, d, e, consts, uid="_A")
            with ExitStack() as ictx:
                _emit_body(nc, tc, ictx, e, d, consts, uid="_B")
        nc.sync.dma_start(out=out_ext[:, :], in_=d[0:P, 0:4])
    nc.compile()
    return nc


_PROGRAMS: dict[bool, bass.Bass] = {}


def _run(inputs: dict, **run_kwargs):
    x = np.ascontiguousarray(np.asarray(inputs["x"], dtype=np.float32))
    w = np.ascontiguousarray(np.asarray(inputs["weight"], dtype=np.float32))
    apply_weight = not bool(np.all(w == 1.0))
    if apply_weight not in _PROGRAMS:
        _PROGRAMS[apply_weight] = _build(apply_weight)
    nc = _PROGRAMS[apply_weight]
    consts = _consts()
    in_maps = []
    for b in range(B):
        m = {"x": x[b], **consts}
        if apply_weight:
            m["weight"] = w
        in_maps.append(m)
    res = run_bass_kernel_spmd(nc, in_maps, core_ids=list(range(B)),
                               **run_kwargs)
    out = np.stack([res.results[b]["out"] for b in range(B)], axis=0)
    return out, res


def kernel(**inputs) -> np.ndarray:
    in_dtype = np.asarray(inputs["x"]).dtype
    out, _ = _run(inputs)
    return out.astype(in_dtype)
